# revision 11
# baseline (speedup 1.0000x reference)
"""Bass/Tile Trainium2 kernel for BinaryMultiHeadAttention (B=2, S=2048, D=1024, H=16).

Sharding: token-parallel across 8 cores. Core c handles batch c//4, tokens
(c%4)*512..+512 of that batch. Q/K/V projections are computed for the core's
own 512 tokens only; K (feature-major) and V (token-major) are AllGathered
within each 4-core batch group so every core sees its batch's full 2048
tokens for attention. The squared-softmax p^2/sum(p^2) is computed exactly as
softmax(2*scores) = exp(dot/4)/sum(exp(dot/4)) (dot is an integer in [0,64],
so no max subtraction is needed). The ones-column trick makes one PV matmul
produce both the attention numerator and the softmax denominator.

Self-contained: hardcodes shapes; builds + compiles the Bass program once per
process and runs it SPMD on cores 0-7.
"""

import numpy as np
import ml_dtypes

B, S, D, H, HD = 2, 2048, 1024, 16, 64
TPC = 512  # tokens per core
NCORES = 8
GROUPS = [[0, 1, 2, 3], [4, 5, 6, 7]]

_CACHE = {}


def _build_program():
    import concourse.mybir as mybir
    import concourse.tile as tile
    from concourse import bacc

    F32 = mybir.dt.float32
    BF16 = mybir.dt.bfloat16
    AF = mybir.ActivationFunctionType
    GT = mybir.AluOpType.is_gt
    MULT = mybir.AluOpType.mult

    nc = bacc.Bacc("TRN2", target_bir_lowering=False, debug=False, num_devices=NCORES)

    xT = nc.dram_tensor("xT", [D, TPC], BF16, kind="ExternalInput")
    wqT = nc.dram_tensor("wqT", [D, D], BF16, kind="ExternalInput")
    wkT = nc.dram_tensor("wkT", [D, D], BF16, kind="ExternalInput")
    wvT = nc.dram_tensor("wvT", [D, D], BF16, kind="ExternalInput")
    woT = nc.dram_tensor("woT", [D, D], BF16, kind="ExternalInput")
    thrq = nc.dram_tensor("thrq", [128, 8], F32, kind="ExternalInput")
    thrk = nc.dram_tensor("thrk", [128, 8], F32, kind="ExternalInput")
    thrvb = nc.dram_tensor("thrvb", [128, D], F32, kind="ExternalInput")
    throb = nc.dram_tensor("throb", [128, D], F32, kind="ExternalInput")
    y = nc.dram_tensor("y", [TPC, D], F32, kind="ExternalOutput")

    with tile.TileContext(nc) as tc:
        with (
            tc.tile_pool(name="w", bufs=2) as wpool,
            tc.tile_pool(name="big", bufs=1) as bigpool,
            tc.tile_pool(name="consts", bufs=1) as cpool,
            tc.tile_pool(name="stage", bufs=4) as stpool,
            tc.tile_pool(name="kt", bufs=2) as ktpool,
            tc.tile_pool(name="vh", bufs=2) as vhpool,
            tc.tile_pool(name="p", bufs=2) as ppool,
            tc.tile_pool(name="nrm", bufs=4) as nrmpool,
            tc.tile_pool(name="bp", bufs=2) as bppool,
            tc.tile_pool(name="yo", bufs=3) as yopool,
            tc.tile_pool(name="dram", bufs=1, space="DRAM") as drpool,
            tc.tile_pool(name="ps_s", bufs=1, space="PSUM") as ps_s,
            tc.tile_pool(name="ps_pv", bufs=2, space="PSUM") as ps_pv,
            tc.tile_pool(name="ps_mm", bufs=2, space="PSUM") as ps_mm,
        ):
            # ---- constants
            thrq_sb = cpool.tile([128, 8], F32, tag="thrq")
            nc.sync.dma_start(thrq_sb[:], thrq[:, :])
            # tiny warmup collective: absorbs the first-collective ncfw/skew
            # cost concurrently with the projection phase
            dummy_in = drpool.tile([128, 8], F32, tag="dumi")
            dummy_out = drpool.tile([512, 8], F32, tag="dumo")
            nc.sync.dma_start(dummy_in[:], thrq[:, :])
            nc.gpsimd.collective_compute(
                "AllGather",
                mybir.AluOpType.bypass,
                replica_groups=GROUPS,
                ins=[dummy_in.opt()],
                outs=[dummy_out.opt()],
            )
            thrk_sb = cpool.tile([128, 8], F32, tag="thrk")
            nc.sync.dma_start(thrk_sb[:], thrk[:, :])
            thrvb_sb = cpool.tile([128, D], F32, tag="thrvb")
            nc.sync.dma_start(thrvb_sb[:], thrvb[:, :])
            throb_sb = cpool.tile([128, D], F32, tag="throb")
            nc.sync.dma_start(throb_sb[:], throb[:, :])
            # ones rows at partition bases 0 and 32 (for the K=1 broadcast
            # matmuls; partition bases must be in {0, 32, 64, 96})
            ones_sb = cpool.tile([33, 64], F32, tag="ones")
            nc.vector.memset(ones_sb[:], 1.0)

            # ---- load x and K/V weights
            xt = bigpool.tile([128, 8, TPC], BF16, tag="xt")
            nc.sync.dma_start(xt[:], xT[:, :].rearrange("(c p) t -> p c t", p=128))
            wk_sb = wpool.tile([128, 8, D], BF16, tag="w")
            nc.sync.dma_start(wk_sb[:], wkT[:, :].rearrange("(c p) f -> p c f", p=128))
            wv_sb = wpool.tile([128, 8, D], BF16, tag="w")
            nc.sync.dma_start(wv_sb[:], wvT[:, :].rearrange("(c p) f -> p c f", p=128))

            # one AG payload: rows 0-1023 = KT_c [1024 f, 512 t]; rows
            # 1024-2047 = V_c [512 t, 1024 f] flattened to [1024, 512]
            ag_in = drpool.tile([2 * D, TPC], BF16, tag="agi")
            ag_out = drpool.tile([8 * D, TPC], BF16, tag="ago")
            v_in_view = ag_in[:, :].rearrange(
                "(half tt p two) t -> half tt two p t", half=2, tt=4, p=128, two=2
            )

            # ---- K projection: KT_c [1024 f, 512 t] binary, feature-major
            for jf in range(8):
                ps = ps_mm.tile([128, 512], F32, tag="mm")
                for dc in range(8):
                    nc.tensor.matmul(
                        ps[:],
                        lhsT=wk_sb[:, dc, jf * 128 : (jf + 1) * 128],
                        rhs=xt[:, dc, :],
                        start=(dc == 0),
                        stop=(dc == 7),
                    )
                st = stpool.tile([128, TPC], BF16, tag="st")
                nc.vector.tensor_scalar(
                    out=st[:],
                    in0=ps[:],
                    scalar1=thrk_sb[:, jf : jf + 1],
                    scalar2=None,
                    op0=GT,
                )
                nc.sync.dma_start(ag_in[jf * 128 : (jf + 1) * 128, :], st[:])

            # ---- V projection: V_c [512 t, 1024 f] binary, token-major
            for tt in range(4):
                for fh in range(2):
                    ps = ps_mm.tile([128, 512], F32, tag="mm")
                    for dc in range(8):
                        nc.tensor.matmul(
                            ps[:],
                            lhsT=xt[:, dc, tt * 128 : (tt + 1) * 128],
                            rhs=wv_sb[:, dc, fh * 512 : (fh + 1) * 512],
                            start=(dc == 0),
                            stop=(dc == 7),
                        )
                    st = stpool.tile([128, TPC], BF16, tag="st")
                    nc.vector.tensor_tensor(
                        out=st[:],
                        in0=ps[:],
                        in1=thrvb_sb[:, fh * 512 : (fh + 1) * 512],
                        op=GT,
                    )
                    nc.sync.dma_start(v_in_view[1, tt, fh, :, :], st[:])

            # ---- AllGather K+V within each batch group of 4 cores
            nc.gpsimd.collective_compute(
                "AllGather",
                mybir.AluOpType.bypass,
                replica_groups=GROUPS,
                ins=[ag_in.opt()],
                outs=[ag_out.opt()],
            )

            # ---- Q projection (overlaps the collectives)
            wq_sb = wpool.tile([128, 8, D], BF16, tag="w")
            nc.sync.dma_start(wq_sb[:], wqT[:, :].rearrange("(c p) f -> p c f", p=128))
            qt = bigpool.tile([128, 8, TPC], BF16, tag="qt")
            for jf in range(8):
                ps = ps_mm.tile([128, 512], F32, tag="mm")
                for dc in range(8):
                    nc.tensor.matmul(
                        ps[:],
                        lhsT=wq_sb[:, dc, jf * 128 : (jf + 1) * 128],
                        rhs=xt[:, dc, :],
                        start=(dc == 0),
                        stop=(dc == 7),
                    )
                nc.vector.tensor_scalar(
                    out=qt[:, jf, :],
                    in0=ps[:],
                    scalar1=thrq_sb[:, jf : jf + 1],
                    scalar2=None,
                    op0=GT,
                )

            wo_sb = wpool.tile([128, 8, D], BF16, tag="w")
            nc.sync.dma_start(wo_sb[:], woT[:, :].rearrange("(c p) f -> p c f", p=128))

            at = bigpool.tile([128, 8, TPC], BF16, tag="at")
            kt_view = ag_out[:, :].rearrange("(r half z) t -> half z r t", r=4, half=2)
            v_out_view = ag_out[:, :].rearrange(
                "(r half lc p two) t -> half p r lc (two t)",
                r=4,
                half=2,
                lc=4,
                p=128,
                two=2,
            )
            # stage full V [2048 tok, 1024 f] into SBUF once (2KB descriptors)
            v_all = bigpool.tile([128, 16, D], BF16, tag="vall")
            for r in range(4):
                nc.sync.dma_start(
                    v_all[:, r * 4 : (r + 1) * 4, :], v_out_view[1, :, r, :, :]
                )

            # ---- attention, head pair jj = heads (2jj, 2jj+1)
            for jj in range(8):
                kt = ktpool.tile([128, 4, TPC], BF16, tag="kt")
                nc.sync.dma_start(kt[:], kt_view[0, jj * 128 : (jj + 1) * 128, :, :])
                pv_tiles = []
                den = nrmpool.tile([33, 512], F32, tag="den")
                for hp in range(2):
                    h = 2 * jj + hp
                    vh = vhpool.tile([128, 16, 65], BF16, tag="vh")
                    nc.vector.memset(vh[:, :, 64:65], 1.0)
                    nc.vector.tensor_copy(
                        vh[:, :, 0:64], v_all[:, :, h * 64 : (h + 1) * 64]
                    )
                    p_t = ppool.tile([128, 16 * 512], BF16, tag="p")
                    for g in range(4):
                        sc = ps_s.tile([128, 2048], F32, tag="sc")
                        for s2 in range(4):
                            kcc = 4 * g + s2
                            r, lc = kcc // 4, kcc % 4
                            nc.tensor.matmul(
                                sc[:, s2 * 512 : (s2 + 1) * 512],
                                lhsT=kt[
                                    hp * 64 : (hp + 1) * 64,
                                    r,
                                    lc * 128 : (lc + 1) * 128,
                                ],
                                rhs=qt[hp * 64 : (hp + 1) * 64, jj, :],
                                start=True,
                                stop=True,
                            )
                        # p = exp(dot/4); squared-renormalized softmax == softmax(2s)
                        nc.scalar.activation(
                            p_t[:, g * 2048 : (g + 1) * 2048],
                            sc[:],
                            AF.Exp,
                            bias=0.0,
                            scale=0.25,
                        )
                    pv = ps_pv.tile([65, 512], F32, tag="pv")
                    for kc in range(16):
                        nc.tensor.matmul(
                            pv[:],
                            lhsT=vh[:, kc, :],
                            rhs=p_t[:, kc * 512 : (kc + 1) * 512],
                            start=(kc == 0),
                            stop=(kc == 15),
                        )
                    nc.vector.tensor_copy(den[32 * hp : 32 * hp + 1, :], pv[64:65, :])
                    pv_tiles.append(pv)
                rec = nrmpool.tile([33, 512], F32, tag="rec")
                nc.vector.reciprocal(rec[:], den[:])
                for hp in range(2):
                    bpp = ps_mm.tile([128, 512], F32, tag="mm")
                    nc.tensor.matmul(
                        bpp[0:64, :],
                        lhsT=ones_sb[32 * hp : 32 * hp + 1, :],
                        rhs=rec[32 * hp : 32 * hp + 1, :],
                        start=True,
                        stop=True,
                    )
                    bps = bppool.tile([64, 512], F32, tag="bp")
                    nc.vector.tensor_copy(bps[:], bpp[0:64, :])
                    nc.vector.tensor_tensor(
                        out=at[hp * 64 : (hp + 1) * 64, jj, :],
                        in0=pv_tiles[hp][0:64, :],
                        in1=bps[:],
                        op=MULT,
                    )

            # ---- output projection + threshold
            for tt in range(4):
                for fh in range(2):
                    ps = ps_mm.tile([128, 512], F32, tag="mm")
                    for jj in range(8):
                        nc.tensor.matmul(
                            ps[:],
                            lhsT=at[:, jj, tt * 128 : (tt + 1) * 128],
                            rhs=wo_sb[:, jj, fh * 512 : (fh + 1) * 512],
                            start=(jj == 0),
                            stop=(jj == 7),
                        )
                    ys = yopool.tile([128, 512], F32, tag="y")
                    nc.vector.tensor_tensor(
                        out=ys[:],
                        in0=ps[:],
                        in1=throb_sb[:, fh * 512 : (fh + 1) * 512],
                        op=GT,
                    )
                    nc.sync.dma_start(
                        y[tt * 128 : (tt + 1) * 128, fh * 512 : (fh + 1) * 512], ys[:]
                    )

    nc.compile()
    return nc


def _get_program():
    if "nc" not in _CACHE:
        _CACHE["nc"] = _build_program()
    return _CACHE["nc"]


def _prep_inputs(x, wq, bq, wk, bk, wv, bv, wo, bo):
    bf16 = ml_dtypes.bfloat16
    x = np.asarray(x, dtype=np.float32)

    def binT(w):
        bw = np.clip(np.sign(np.asarray(w, dtype=np.float32)), 0.0, 1.0)
        return np.ascontiguousarray(bw.T).astype(bf16)

    shared = {
        "wqT": binT(wq),
        "wkT": binT(wk),
        "wvT": binT(wv),
        "woT": binT(wo),
        "thrq": np.ascontiguousarray(
            (0.5 - np.asarray(bq, np.float32)).reshape(8, 128).T
        ),
        "thrk": np.ascontiguousarray(
            (0.5 - np.asarray(bk, np.float32)).reshape(8, 128).T
        ),
        "thrvb": np.ascontiguousarray(
            np.tile((0.5 - np.asarray(bv, np.float32))[None, :], (128, 1))
        ),
        "throb": np.ascontiguousarray(
            np.tile((0.5 - np.asarray(bo, np.float32))[None, :], (128, 1))
        ),
    }
    in_maps = []
    for c in range(NCORES):
        b, blk = c // 4, c % 4
        xT_c = np.ascontiguousarray(x[b, blk * TPC : (blk + 1) * TPC, :].T).astype(bf16)
        m = dict(shared)
        m["xT"] = xT_c
        in_maps.append(m)
    return in_maps


def _gather_output(results):
    y = np.empty((B, S, D), dtype=np.float32)
    for c in range(NCORES):
        b, blk = c // 4, c % 4
        y[b, blk * TPC : (blk + 1) * TPC, :] = results[c]["y"]
    return y


def _run(in_maps, **kw):
    from concourse.bass_utils import run_bass_kernel_spmd

    nc = _get_program()
    return run_bass_kernel_spmd(nc, in_maps, list(range(NCORES)), **kw)


def kernel(x, wq, bq, wk, bk, wv, bv, wo, bo):
    in_maps = _prep_inputs(x, wq, bq, wk, bk, wv, bv, wo, bo)
    res = _run(in_maps)
    return _gather_output(res.results)


def run_traced(inputs, **kw):
    """For test.py: run with NTFF tracing, return (output, BassKernelResults)."""
    in_maps = _prep_inputs(**inputs)
    res = _run(in_maps, trace=True, **kw)
    return _gather_output(res.results), res


# revision 12
# speedup vs baseline: 1.3442x; 1.3442x over previous
"""Bass/Tile Trainium2 kernel for BinaryMultiHeadAttention (B=2, S=2048, D=1024, H=16).

Sharding: token-parallel across 8 cores with zero collectives. Core c handles
batch c//4, query tokens (c%4)*512..+512. Each core redundantly computes the
binary K/V projections for its batch's full 2048 tokens (cheap fp8 matmuls,
everything stays in SBUF), so attention needs no cross-core communication and
the kernel has no cross-core sync points.

Math notes: the reference's squared-softmax p^2/sum(p^2) equals
softmax(2*scores) = exp(dot/4)/sum(exp(dot/4)) exactly (up to the +1e-8),
and dot is an integer in [0, 64] so no max-subtraction is needed. Binary
projections are computed as is_gt(x @ w^T, 0.5 - bias) with the bias folded
into per-feature thresholds. A ones-column appended to V makes one PV matmul
accumulation produce both the attention numerator and the softmax
denominator.

Self-contained: hardcodes shapes; builds + compiles the Bass program once per
process and runs it SPMD on cores 0-7.
"""

import numpy as np
import ml_dtypes

B, S, D, H, HD = 2, 2048, 1024, 16, 64
TPC = 512  # query tokens per core
NCORES = 8

_CACHE = {}


def _build_program():
    import concourse.mybir as mybir
    import concourse.tile as tile
    from concourse import bacc

    F32 = mybir.dt.float32
    BF16 = mybir.dt.bfloat16
    FP8 = mybir.dt.float8e4
    AF = mybir.ActivationFunctionType
    GT = mybir.AluOpType.is_gt
    MULT = mybir.AluOpType.mult

    nc = bacc.Bacc("TRN2", target_bir_lowering=False, debug=False, num_devices=NCORES)

    # full-batch x^T and own-shard x^T (host-prepared, fp8)
    xT = nc.dram_tensor("xT", [D, S], FP8, kind="ExternalInput")
    xqT = nc.dram_tensor("xqT", [D, TPC], FP8, kind="ExternalInput")
    wqT = nc.dram_tensor("wqT", [D, D], FP8, kind="ExternalInput")
    wkT = nc.dram_tensor("wkT", [D, D], FP8, kind="ExternalInput")
    wvT = nc.dram_tensor("wvT", [D, D], FP8, kind="ExternalInput")
    woT = nc.dram_tensor("woT", [D, D], BF16, kind="ExternalInput")
    thrq = nc.dram_tensor("thrq", [128, 8], F32, kind="ExternalInput")
    thrk = nc.dram_tensor("thrk", [128, 8], F32, kind="ExternalInput")
    thrvb = nc.dram_tensor("thrvb", [128, D], F32, kind="ExternalInput")
    throb = nc.dram_tensor("throb", [128, D], F32, kind="ExternalInput")
    y = nc.dram_tensor("y", [TPC, D], F32, kind="ExternalOutput")

    with tile.TileContext(nc) as tc:
        with (
            tc.tile_pool(name="w", bufs=2) as wpool,
            tc.tile_pool(name="wo", bufs=1) as wopool,
            tc.tile_pool(name="big", bufs=1) as bigpool,
            tc.tile_pool(name="consts", bufs=1) as cpool,
            tc.tile_pool(name="vh", bufs=2) as vhpool,
            tc.tile_pool(name="p", bufs=12) as ppool,
            tc.tile_pool(name="nrm", bufs=4) as nrmpool,
            tc.tile_pool(name="bp", bufs=2) as bppool,
            tc.tile_pool(name="yo", bufs=3) as yopool,
            tc.tile_pool(name="ps_s", bufs=2, space="PSUM") as ps_s,
            tc.tile_pool(name="ps_pv", bufs=2, space="PSUM") as ps_pv,
            tc.tile_pool(name="ps_mm", bufs=2, space="PSUM") as ps_mm,
        ):
            # ---- constants
            thrq_sb = cpool.tile([128, 8], F32, tag="thrq")
            nc.sync.dma_start(thrq_sb[:], thrq[:, :])
            thrk_sb = cpool.tile([128, 8], F32, tag="thrk")
            nc.sync.dma_start(thrk_sb[:], thrk[:, :])
            thrvb_sb = cpool.tile([128, D], F32, tag="thrvb")
            nc.sync.dma_start(thrvb_sb[:], thrvb[:, :])
            throb_sb = cpool.tile([128, D], F32, tag="throb")
            nc.sync.dma_start(throb_sb[:], throb[:, :])
            # ones rows at partition bases 0 and 32 for the K=1 broadcast mms
            ones_sb = cpool.tile([33, 64], F32, tag="ones")
            nc.vector.memset(ones_sb[:], 1.0)

            # ---- load x (full batch + own shard) and weights
            xt = bigpool.tile([128, 8, S], FP8, tag="xt")
            nc.sync.dma_start(xt[:], xT[:, :].rearrange("(c p) t -> p c t", p=128))
            xqt = bigpool.tile([128, 8, TPC], FP8, tag="xqt")
            nc.sync.dma_start(xqt[:], xqT[:, :].rearrange("(c p) t -> p c t", p=128))
            wk_sb = wpool.tile([128, 8, D], FP8, tag="w")
            nc.sync.dma_start(wk_sb[:], wkT[:, :].rearrange("(c p) f -> p c f", p=128))
            wv_sb = wpool.tile([128, 8, D], FP8, tag="w")
            nc.sync.dma_start(wv_sb[:], wvT[:, :].rearrange("(c p) f -> p c f", p=128))

            kt_full = bigpool.tile([128, 8, S], FP8, tag="kt")
            v_all = bigpool.tile([128, 16, D], BF16, tag="vall")

            # ---- K projection, full batch: KT [1024 f, 2048 t] binary fp8
            for jf in range(8):
                for tch in range(4):
                    ps = ps_mm.tile([128, 512], F32, tag="mm")
                    for dc in range(8):
                        nc.tensor.matmul(
                            ps[:],
                            lhsT=wk_sb[:, dc, jf * 128 : (jf + 1) * 128],
                            rhs=xt[:, dc, tch * 512 : (tch + 1) * 512],
                            start=(dc == 0),
                            stop=(dc == 7),
                        )
                    nc.vector.tensor_scalar(
                        out=kt_full[:, jf, tch * 512 : (tch + 1) * 512],
                        in0=ps[:],
                        scalar1=thrk_sb[:, jf : jf + 1],
                        scalar2=None,
                        op0=GT,
                    )

            # ---- V projection, full batch: V [2048 t, 1024 f] binary bf16
            for tt in range(16):
                for fh in range(2):
                    ps = ps_mm.tile([128, 512], F32, tag="mm")
                    for dc in range(8):
                        nc.tensor.matmul(
                            ps[:],
                            lhsT=xt[:, dc, tt * 128 : (tt + 1) * 128],
                            rhs=wv_sb[:, dc, fh * 512 : (fh + 1) * 512],
                            start=(dc == 0),
                            stop=(dc == 7),
                        )
                    nc.vector.tensor_tensor(
                        out=v_all[:, tt, fh * 512 : (fh + 1) * 512],
                        in0=ps[:],
                        in1=thrvb_sb[:, fh * 512 : (fh + 1) * 512],
                        op=GT,
                    )

            # ---- Q projection (own 512 tokens): QT [1024 f, 512 t] binary fp8
            wq_sb = wpool.tile([128, 8, D], FP8, tag="w")
            nc.sync.dma_start(wq_sb[:], wqT[:, :].rearrange("(c p) f -> p c f", p=128))
            qt = bigpool.tile([128, 8, TPC], FP8, tag="qt")
            for jf in range(8):
                ps = ps_mm.tile([128, 512], F32, tag="mm")
                for dc in range(8):
                    nc.tensor.matmul(
                        ps[:],
                        lhsT=wq_sb[:, dc, jf * 128 : (jf + 1) * 128],
                        rhs=xqt[:, dc, :],
                        start=(dc == 0),
                        stop=(dc == 7),
                    )
                nc.vector.tensor_scalar(
                    out=qt[:, jf, :],
                    in0=ps[:],
                    scalar1=thrq_sb[:, jf : jf + 1],
                    scalar2=None,
                    op0=GT,
                )

            wo_sb = wopool.tile([128, 8, D], BF16, tag="wo")
            nc.sync.dma_start(wo_sb[:], woT[:, :].rearrange("(c p) f -> p c f", p=128))

            at = bigpool.tile([128, 8, TPC], BF16, tag="at")

            # ---- attention; head pair jj = heads (2jj, 2jj+1)
            for jj in range(8):
                pv_tiles = []
                den = nrmpool.tile([33, 512], F32, tag="den")
                for hp in range(2):
                    h = 2 * jj + hp
                    vh = vhpool.tile([128, 16, 65], BF16, tag="vh")
                    nc.vector.memset(vh[:, :, 64:65], 1.0)
                    nc.vector.tensor_copy(
                        vh[:, :, 0:64], v_all[:, :, h * 64 : (h + 1) * 64]
                    )
                    p_tiles = []
                    for g in range(8):  # score groups of 2 ktiles, dbl-buffered
                        sc = ps_s.tile([128, 1024], F32, tag="sc")
                        for s2 in range(2):
                            kcc = 2 * g + s2
                            nc.tensor.matmul(
                                sc[:, s2 * 512 : (s2 + 1) * 512],
                                lhsT=kt_full[
                                    hp * 64 : (hp + 1) * 64,
                                    jj,
                                    kcc * 128 : (kcc + 1) * 128,
                                ],
                                rhs=qt[hp * 64 : (hp + 1) * 64, jj, :],
                                start=True,
                                stop=True,
                            )
                        # p = exp(dot/4); squared-softmax == softmax(2s)
                        p_g = ppool.tile([128, 1024], BF16, tag="p")
                        nc.scalar.activation(
                            p_g[:], sc[:], AF.Exp, bias=0.0, scale=0.25
                        )
                        p_tiles.append(p_g)
                    pv = ps_pv.tile([65, 512], F32, tag="pv")
                    for kc in range(16):
                        nc.tensor.matmul(
                            pv[:],
                            lhsT=vh[:, kc, :],
                            rhs=p_tiles[kc // 2][:, (kc % 2) * 512 : (kc % 2 + 1) * 512],
                            start=(kc == 0),
                            stop=(kc == 15),
                        )
                    nc.vector.tensor_copy(den[32 * hp : 32 * hp + 1, :], pv[64:65, :])
                    pv_tiles.append(pv)
                rec = nrmpool.tile([33, 512], F32, tag="rec")
                nc.vector.reciprocal(rec[:], den[:])
                for hp in range(2):
                    bpp = ps_mm.tile([128, 512], F32, tag="mm")
                    nc.tensor.matmul(
                        bpp[0:64, :],
                        lhsT=ones_sb[32 * hp : 32 * hp + 1, :],
                        rhs=rec[32 * hp : 32 * hp + 1, :],
                        start=True,
                        stop=True,
                    )
                    bps = bppool.tile([64, 512], F32, tag="bp")
                    nc.vector.tensor_copy(bps[:], bpp[0:64, :])
                    nc.vector.tensor_tensor(
                        out=at[hp * 64 : (hp + 1) * 64, jj, :],
                        in0=pv_tiles[hp][0:64, :],
                        in1=bps[:],
                        op=MULT,
                    )

            # ---- output projection + threshold
            for tt in range(4):
                for fh in range(2):
                    ps = ps_mm.tile([128, 512], F32, tag="mm")
                    for jj in range(8):
                        nc.tensor.matmul(
                            ps[:],
                            lhsT=at[:, jj, tt * 128 : (tt + 1) * 128],
                            rhs=wo_sb[:, jj, fh * 512 : (fh + 1) * 512],
                            start=(jj == 0),
                            stop=(jj == 7),
                        )
                    ys = yopool.tile([128, 512], F32, tag="y")
                    nc.vector.tensor_tensor(
                        out=ys[:],
                        in0=ps[:],
                        in1=throb_sb[:, fh * 512 : (fh + 1) * 512],
                        op=GT,
                    )
                    nc.sync.dma_start(
                        y[tt * 128 : (tt + 1) * 128, fh * 512 : (fh + 1) * 512], ys[:]
                    )

    nc.compile()
    return nc


def _get_program():
    if "nc" not in _CACHE:
        _CACHE["nc"] = _build_program()
    return _CACHE["nc"]


def _prep_inputs(x, wq, bq, wk, bk, wv, bv, wo, bo):
    bf16 = ml_dtypes.bfloat16
    fp8 = ml_dtypes.float8_e4m3
    x = np.asarray(x, dtype=np.float32)

    def binT(w, dt):
        bw = np.clip(np.sign(np.asarray(w, dtype=np.float32)), 0.0, 1.0)
        return np.ascontiguousarray(bw.T).astype(dt)

    shared = {
        "wqT": binT(wq, fp8),
        "wkT": binT(wk, fp8),
        "wvT": binT(wv, fp8),
        "woT": binT(wo, bf16),
        "thrq": np.ascontiguousarray(
            (0.5 - np.asarray(bq, np.float32)).reshape(8, 128).T
        ),
        "thrk": np.ascontiguousarray(
            (0.5 - np.asarray(bk, np.float32)).reshape(8, 128).T
        ),
        "thrvb": np.ascontiguousarray(
            np.tile((0.5 - np.asarray(bv, np.float32))[None, :], (128, 1))
        ),
        "throb": np.ascontiguousarray(
            np.tile((0.5 - np.asarray(bo, np.float32))[None, :], (128, 1))
        ),
    }
    in_maps = []
    for c in range(NCORES):
        b, blk = c // 4, c % 4
        xT_b = np.ascontiguousarray(x[b].T).astype(fp8)
        m = dict(shared)
        m["xT"] = xT_b
        m["xqT"] = np.ascontiguousarray(xT_b[:, blk * TPC : (blk + 1) * TPC])
        in_maps.append(m)
    return in_maps


def _gather_output(results):
    y = np.empty((B, S, D), dtype=np.float32)
    for c in range(NCORES):
        b, blk = c // 4, c % 4
        y[b, blk * TPC : (blk + 1) * TPC, :] = results[c]["y"]
    return y


def _run(in_maps, **kw):
    from concourse.bass_utils import run_bass_kernel_spmd

    nc = _get_program()
    return run_bass_kernel_spmd(nc, in_maps, list(range(NCORES)), **kw)


def kernel(x, wq, bq, wk, bk, wv, bv, wo, bo):
    in_maps = _prep_inputs(x, wq, bq, wk, bk, wv, bv, wo, bo)
    res = _run(in_maps)
    return _gather_output(res.results)


def run_traced(inputs, **kw):
    """For test.py: run with NTFF tracing, return (output, BassKernelResults)."""
    in_maps = _prep_inputs(**inputs)
    res = _run(in_maps, trace=True, **kw)
    return _gather_output(res.results), res


# revision 20
# speedup vs baseline: 1.6147x; 1.2012x over previous
"""Bass/Tile Trainium2 kernel for BinaryMultiHeadAttention (B=2, S=2048, D=1024, H=16).

Sharding: token-parallel across 8 cores with zero collectives. Core c handles
batch c//4, query tokens (c%4)*512..+512. Each core redundantly computes the
binary K/V projections for its batch's full 2048 tokens (cheap fp8 matmuls,
everything stays in SBUF), so attention needs no cross-core communication and
the kernel has no cross-core sync points.

Math notes: the reference's squared-softmax p^2/sum(p^2) equals
softmax(2*scores) = exp(dot/4)/sum(exp(dot/4)) exactly (up to the +1e-8),
and dot is an integer in [0, 64] so no max-subtraction is needed. Binary
projections are computed as is_gt(x @ w^T, 0.5 - bias) with the bias folded
into per-feature thresholds. A ones-column appended to V makes one PV matmul
accumulation produce both the attention numerator and the softmax
denominator.

Self-contained: hardcodes shapes; builds + compiles the Bass program once per
process and runs it SPMD on cores 0-7.
"""

import numpy as np
import ml_dtypes

B, S, D, H, HD = 2, 2048, 1024, 16, 64
TPC = 512  # query tokens per core
NCORES = 8

_CACHE = {}


def _build_program():
    import concourse.mybir as mybir
    import concourse.tile as tile
    from concourse import bacc

    F32 = mybir.dt.float32
    BF16 = mybir.dt.bfloat16
    FP8 = mybir.dt.float8e4
    FP8E5 = mybir.dt.float8e5
    AF = mybir.ActivationFunctionType
    GT = mybir.AluOpType.is_gt
    MULT = mybir.AluOpType.mult
    DR = mybir.MatmulPerfMode.DoubleRow

    nc = bacc.Bacc("TRN2", target_bir_lowering=False, debug=False, num_devices=NCORES)

    # full-batch x^T and own-shard x^T (host-prepared, fp8)
    xT = nc.dram_tensor("xT", [D, S], FP8, kind="ExternalInput")
    xqT = nc.dram_tensor("xqT", [D, TPC], FP8, kind="ExternalInput")
    wqT = nc.dram_tensor("wqT", [D, D], FP8, kind="ExternalInput")
    wkT = nc.dram_tensor("wkT", [D, D], FP8, kind="ExternalInput")
    wvT = nc.dram_tensor("wvT", [D, D], FP8, kind="ExternalInput")
    woT = nc.dram_tensor("woT", [D, D], FP8, kind="ExternalInput")
    thrq = nc.dram_tensor("thrq", [128, 8], F32, kind="ExternalInput")
    thrk = nc.dram_tensor("thrk", [128, 8], F32, kind="ExternalInput")
    thrvb = nc.dram_tensor("thrvb", [128, D], F32, kind="ExternalInput")
    throb = nc.dram_tensor("throb", [128, D], F32, kind="ExternalInput")
    y = nc.dram_tensor("y", [TPC, D], F32, kind="ExternalOutput")

    with tile.TileContext(nc) as tc:
        with (
            tc.tile_pool(name="w", bufs=2) as wpool,
            tc.tile_pool(name="wo", bufs=1) as wopool,
            tc.tile_pool(name="big", bufs=1) as bigpool,
            tc.tile_pool(name="consts", bufs=1) as cpool,
            tc.tile_pool(name="vh", bufs=2) as vhpool,
            tc.tile_pool(name="p", bufs=12) as ppool,
            tc.tile_pool(name="nrm", bufs=4) as nrmpool,
            tc.tile_pool(name="bp", bufs=2) as bppool,
            tc.tile_pool(name="yo", bufs=3) as yopool,
            tc.tile_pool(name="ps_s", bufs=2, space="PSUM") as ps_s,
            tc.tile_pool(name="ps_pv", bufs=2, space="PSUM") as ps_pv,
            tc.tile_pool(name="ps_mm", bufs=2, space="PSUM") as ps_mm,
        ):
            # ---- constants
            thrq_sb = cpool.tile([128, 8], F32, tag="thrq")
            nc.sync.dma_start(thrq_sb[:], thrq[:, :])
            thrk_sb = cpool.tile([128, 8], F32, tag="thrk")
            nc.sync.dma_start(thrk_sb[:], thrk[:, :])
            thrvb_sb = cpool.tile([128, D], F32, tag="thrvb")
            nc.sync.dma_start(thrvb_sb[:], thrvb[:, :])
            throb_sb = cpool.tile([128, D], F32, tag="throb")
            nc.sync.dma_start(throb_sb[:], throb[:, :])
            # ones rows at partition bases 0 and 32 for the K=1 broadcast mms
            ones_sb = cpool.tile([33, 64], F32, tag="ones")
            nc.vector.memset(ones_sb[:], 1.0)
            nbias = cpool.tile([128, 1], F32, tag="nbias")
            nc.vector.memset(nbias[:], -6.0)

            # ---- load x (full batch + own shard) and weights, chunked so the
            # first projection matmuls can start before the full load lands
            xt = bigpool.tile([128, 8, S], FP8, tag="xt")
            xt_view = xT[:, :].rearrange("(c p) t -> p c t", p=128)
            wk_sb = wpool.tile([128, 8, D], FP8, tag="w")
            wk_view = wkT[:, :].rearrange("(c p) f -> p c f", p=128)
            for c2 in range(4):
                nc.sync.dma_start(
                    xt[:, 2 * c2 : 2 * c2 + 2, :], xt_view[:, 2 * c2 : 2 * c2 + 2, :]
                )
                nc.sync.dma_start(
                    wk_sb[:, 2 * c2 : 2 * c2 + 2, :],
                    wk_view[:, 2 * c2 : 2 * c2 + 2, :],
                )
            xqt = bigpool.tile([128, 8, TPC], FP8, tag="xqt")
            nc.sync.dma_start(xqt[:], xqT[:, :].rearrange("(c p) t -> p c t", p=128))
            wv_sb = wpool.tile([128, 8, D], FP8, tag="w")
            nc.sync.dma_start(wv_sb[:], wvT[:, :].rearrange("(c p) f -> p c f", p=128))

            kt_full = bigpool.tile([128, 8, S], FP8, tag="kt")
            v_all = bigpool.tile([128, 16, D], FP8E5, tag="vall")

            # ---- K projection, full batch: KT [1024 f, 2048 t] binary fp8
            # (fp8 DoubleRow: contract 256 per matmul via chunk pairs)
            for jf in range(8):
                for tch in range(4):
                    ps = ps_mm.tile([128, 512], F32, tag="mm")
                    for dc in range(4):
                        nc.tensor.matmul(
                            ps[:],
                            lhsT=wk_sb[:, 2 * dc : 2 * dc + 2, jf * 128 : (jf + 1) * 128],
                            rhs=xt[:, 2 * dc : 2 * dc + 2, tch * 512 : (tch + 1) * 512],
                            start=(dc == 0),
                            stop=(dc == 3),
                            perf_mode=DR,
                        )
                    nc.vector.tensor_scalar(
                        out=kt_full[:, jf, tch * 512 : (tch + 1) * 512],
                        in0=ps[:],
                        scalar1=thrk_sb[:, jf : jf + 1],
                        scalar2=None,
                        op0=GT,
                    )

            # ---- V projection, full batch: V [2048 t, 1024 f] binary fp8e5
            for tt in range(16):
                for fh in range(2):
                    ps = ps_mm.tile([128, 512], F32, tag="mm")
                    for dc in range(4):
                        nc.tensor.matmul(
                            ps[:],
                            lhsT=xt[:, 2 * dc : 2 * dc + 2, tt * 128 : (tt + 1) * 128],
                            rhs=wv_sb[:, 2 * dc : 2 * dc + 2, fh * 512 : (fh + 1) * 512],
                            start=(dc == 0),
                            stop=(dc == 3),
                            perf_mode=DR,
                        )
                    nc.vector.tensor_tensor(
                        out=v_all[:, tt, fh * 512 : (fh + 1) * 512],
                        in0=ps[:],
                        in1=thrvb_sb[:, fh * 512 : (fh + 1) * 512],
                        op=GT,
                    )

            # ---- Q projection (own 512 tokens): QT [1024 f, 512 t] binary fp8
            wq_sb = wpool.tile([128, 8, D], FP8, tag="w")
            nc.sync.dma_start(wq_sb[:], wqT[:, :].rearrange("(c p) f -> p c f", p=128))
            qt = bigpool.tile([128, 8, TPC], FP8, tag="qt")
            for jf in range(8):
                ps = ps_mm.tile([128, 512], F32, tag="mm")
                for dc in range(4):
                    nc.tensor.matmul(
                        ps[:],
                        lhsT=wq_sb[:, 2 * dc : 2 * dc + 2, jf * 128 : (jf + 1) * 128],
                        rhs=xqt[:, 2 * dc : 2 * dc + 2, :],
                        start=(dc == 0),
                        stop=(dc == 3),
                        perf_mode=DR,
                    )
                nc.vector.tensor_scalar(
                    out=qt[:, jf, :],
                    in0=ps[:],
                    scalar1=thrq_sb[:, jf : jf + 1],
                    scalar2=None,
                    op0=GT,
                )

            wo_sb = wopool.tile([128, 8, D], FP8, tag="wo")
            nc.sync.dma_start(wo_sb[:], woT[:, :].rearrange("(c p) f -> p c f", p=128))

            at = bigpool.tile([128, 8, TPC], FP8, tag="at")

            # ---- attention; head pair jj = heads (2jj, 2jj+1)
            for jj in range(8):
                pv_tiles = []
                den = nrmpool.tile([33, 512], F32, tag="den")
                for hp in range(2):
                    h = 2 * jj + hp
                    # V' per head: 64 value cols + ones col, padded to stride
                    # 80 (DoubleRow needs middle-dim step % 16 == 0)
                    vh = vhpool.tile([128, 16, 80], FP8E5, tag="vh")
                    nc.vector.memset(vh[:, :, 64:65], 1.0)
                    nc.vector.tensor_copy(
                        vh[:, :, 0:64], v_all[:, :, h * 64 : (h + 1) * 64]
                    )
                    p_tiles = []
                    for g in range(8):  # score groups of 2 ktiles, dbl-buffered
                        sc = ps_s.tile([128, 1024], F32, tag="sc")
                        for s2 in range(2):
                            kcc = 2 * g + s2
                            nc.tensor.matmul(
                                sc[:, s2 * 512 : (s2 + 1) * 512],
                                lhsT=kt_full[
                                    hp * 64 : (hp + 1) * 64,
                                    jj,
                                    kcc * 128 : (kcc + 1) * 128,
                                ],
                                rhs=qt[hp * 64 : (hp + 1) * 64, jj, :],
                                start=True,
                                stop=True,
                            )
                        # p = exp(dot/4 - 6); squared-softmax == softmax(2s),
                        # the e^-6 shift cancels in the normalization and
                        # keeps p inside fp8e5's range [2^-16, 57344]
                        p_g = ppool.tile([128, 2, 512], FP8E5, tag="p")
                        nc.scalar.activation(
                            p_g[:, :, :].rearrange("p a b -> p (a b)"),
                            sc[:],
                            AF.Exp,
                            bias=nbias[:],
                            scale=0.25,
                        )
                        p_tiles.append(p_g)
                    pv = ps_pv.tile([65, 512], F32, tag="pv")
                    for m in range(8):
                        nc.tensor.matmul(
                            pv[:],
                            lhsT=vh[:, 2 * m : 2 * m + 2, 0:65],
                            rhs=p_tiles[m][:],
                            start=(m == 0),
                            stop=(m == 7),
                            perf_mode=DR,
                        )
                    nc.vector.tensor_copy(den[32 * hp : 32 * hp + 1, :], pv[64:65, :])
                    pv_tiles.append(pv)
                rec = nrmpool.tile([33, 512], F32, tag="rec")
                nc.vector.reciprocal(rec[:], den[:])
                for hp in range(2):
                    bpp = ps_mm.tile([128, 512], F32, tag="mm")
                    nc.tensor.matmul(
                        bpp[0:64, :],
                        lhsT=ones_sb[32 * hp : 32 * hp + 1, :],
                        rhs=rec[32 * hp : 32 * hp + 1, :],
                        start=True,
                        stop=True,
                    )
                    bps = bppool.tile([64, 512], F32, tag="bp")
                    nc.vector.tensor_copy(bps[:], bpp[0:64, :])
                    nc.vector.tensor_tensor(
                        out=at[hp * 64 : (hp + 1) * 64, jj, :],
                        in0=pv_tiles[hp][0:64, :],
                        in1=bps[:],
                        op=MULT,
                    )

            # ---- output projection + threshold
            for tt in range(4):
                for fh in range(2):
                    ps = ps_mm.tile([128, 512], F32, tag="mm")
                    for j2 in range(4):
                        nc.tensor.matmul(
                            ps[:],
                            lhsT=at[:, 2 * j2 : 2 * j2 + 2, tt * 128 : (tt + 1) * 128],
                            rhs=wo_sb[:, 2 * j2 : 2 * j2 + 2, fh * 512 : (fh + 1) * 512],
                            start=(j2 == 0),
                            stop=(j2 == 3),
                            perf_mode=DR,
                        )
                    ys = yopool.tile([128, 512], F32, tag="y")
                    nc.vector.tensor_tensor(
                        out=ys[:],
                        in0=ps[:],
                        in1=throb_sb[:, fh * 512 : (fh + 1) * 512],
                        op=GT,
                    )
                    nc.sync.dma_start(
                        y[tt * 128 : (tt + 1) * 128, fh * 512 : (fh + 1) * 512], ys[:]
                    )

    nc.compile()
    return nc


def _get_program():
    if "nc" not in _CACHE:
        _CACHE["nc"] = _build_program()
    return _CACHE["nc"]


def _prep_inputs(x, wq, bq, wk, bk, wv, bv, wo, bo):
    bf16 = ml_dtypes.bfloat16
    fp8 = ml_dtypes.float8_e4m3
    x = np.asarray(x, dtype=np.float32)

    def binT(w, dt):
        bw = np.clip(np.sign(np.asarray(w, dtype=np.float32)), 0.0, 1.0)
        return np.ascontiguousarray(bw.T).astype(dt)

    shared = {
        "wqT": binT(wq, fp8),
        "wkT": binT(wk, fp8),
        "wvT": binT(wv, fp8),
        "woT": binT(wo, fp8),
        "thrq": np.ascontiguousarray(
            (0.5 - np.asarray(bq, np.float32)).reshape(8, 128).T
        ),
        "thrk": np.ascontiguousarray(
            (0.5 - np.asarray(bk, np.float32)).reshape(8, 128).T
        ),
        "thrvb": np.ascontiguousarray(
            np.tile((0.5 - np.asarray(bv, np.float32))[None, :], (128, 1))
        ),
        "throb": np.ascontiguousarray(
            np.tile((0.5 - np.asarray(bo, np.float32))[None, :], (128, 1))
        ),
    }
    in_maps = []
    for c in range(NCORES):
        b, blk = c // 4, c % 4
        xT_b = np.ascontiguousarray(x[b].T).astype(fp8)
        m = dict(shared)
        m["xT"] = xT_b
        m["xqT"] = np.ascontiguousarray(xT_b[:, blk * TPC : (blk + 1) * TPC])
        in_maps.append(m)
    return in_maps


def _gather_output(results):
    y = np.empty((B, S, D), dtype=np.float32)
    for c in range(NCORES):
        b, blk = c // 4, c % 4
        y[b, blk * TPC : (blk + 1) * TPC, :] = results[c]["y"]
    return y


def _run(in_maps, **kw):
    from concourse.bass_utils import run_bass_kernel_spmd

    nc = _get_program()
    return run_bass_kernel_spmd(nc, in_maps, list(range(NCORES)), **kw)


def kernel(x, wq, bq, wk, bk, wv, bv, wo, bo):
    in_maps = _prep_inputs(x, wq, bq, wk, bk, wv, bv, wo, bo)
    res = _run(in_maps)
    return _gather_output(res.results)


def run_traced(inputs, **kw):
    """For test.py: run with NTFF tracing, return (output, BassKernelResults)."""
    in_maps = _prep_inputs(**inputs)
    res = _run(in_maps, trace=True, **kw)
    return _gather_output(res.results), res


# revision 24
# speedup vs baseline: 1.7541x; 1.0863x over previous
"""Bass/Tile Trainium2 kernel for BinaryMultiHeadAttention (B=2, S=2048, D=1024, H=16).

Sharding: token-parallel across 8 cores with zero collectives. Core c handles
batch c//4, query tokens (c%4)*512..+512. Each core redundantly computes the
binary K/V projections for its batch's full 2048 tokens (cheap fp8 matmuls,
everything stays in SBUF), so attention needs no cross-core communication and
the kernel has no cross-core sync points.

Math notes: the reference's squared-softmax p^2/sum(p^2) equals
softmax(2*scores) = exp(dot/4)/sum(exp(dot/4)) exactly (up to the +1e-8),
and dot is an integer in [0, 64] so no max-subtraction is needed. Binary
projections are computed as is_gt(x @ w^T, 0.5 - bias) with the bias folded
into per-feature thresholds. A ones-column appended to V makes one PV matmul
accumulation produce both the attention numerator and the softmax
denominator.

Self-contained: hardcodes shapes; builds + compiles the Bass program once per
process and runs it SPMD on cores 0-7.
"""

import numpy as np
import ml_dtypes

B, S, D, H, HD = 2, 2048, 1024, 16, 64
TPC = 512  # query tokens per core
NCORES = 8

_CACHE = {}


def _build_program():
    import concourse.mybir as mybir
    import concourse.tile as tile
    from concourse import bacc

    F32 = mybir.dt.float32
    BF16 = mybir.dt.bfloat16
    FP8 = mybir.dt.float8e4
    FP8E5 = mybir.dt.float8e5
    AF = mybir.ActivationFunctionType
    GT = mybir.AluOpType.is_gt
    MULT = mybir.AluOpType.mult
    DR = mybir.MatmulPerfMode.DoubleRow

    nc = bacc.Bacc("TRN2", target_bir_lowering=False, debug=False, num_devices=NCORES)

    # full-batch x^T and own-shard x^T (host-prepared, fp8)
    xT = nc.dram_tensor("xT", [D, S], FP8, kind="ExternalInput")
    xqT = nc.dram_tensor("xqT", [D, TPC], FP8, kind="ExternalInput")
    wqT = nc.dram_tensor("wqT", [D, D], FP8, kind="ExternalInput")
    wkT = nc.dram_tensor("wkT", [D, D], FP8, kind="ExternalInput")
    wvT = nc.dram_tensor("wvT", [D, D], FP8, kind="ExternalInput")
    woT = nc.dram_tensor("woT", [D, D], FP8, kind="ExternalInput")
    thrq = nc.dram_tensor("thrq", [128, 8], F32, kind="ExternalInput")
    thrk = nc.dram_tensor("thrk", [128, 8], F32, kind="ExternalInput")
    thrvb = nc.dram_tensor("thrvb", [128, D], F32, kind="ExternalInput")
    throb = nc.dram_tensor("throb", [128, D], F32, kind="ExternalInput")
    y = nc.dram_tensor("y", [TPC, D], F32, kind="ExternalOutput")

    with tile.TileContext(nc) as tc:
        with (
            tc.tile_pool(name="w", bufs=2) as wpool,
            tc.tile_pool(name="wo", bufs=1) as wopool,
            tc.tile_pool(name="big", bufs=1) as bigpool,
            tc.tile_pool(name="consts", bufs=1) as cpool,
            tc.tile_pool(name="vh", bufs=2) as vhpool,
            tc.tile_pool(name="p", bufs=12) as ppool,
            tc.tile_pool(name="nrm", bufs=4) as nrmpool,
            tc.tile_pool(name="bp", bufs=2) as bppool,
            tc.tile_pool(name="au", bufs=4) as aupool,
            tc.tile_pool(name="yo", bufs=3) as yopool,
            tc.tile_pool(name="ps_s", bufs=2, space="PSUM") as ps_s,
            tc.tile_pool(name="ps_pv", bufs=2, space="PSUM") as ps_pv,
            tc.tile_pool(name="ps_mm", bufs=2, space="PSUM") as ps_mm,
        ):
            # ---- constants
            thrq_sb = cpool.tile([128, 8], F32, tag="thrq")
            nc.sync.dma_start(thrq_sb[:], thrq[:, :])
            thrk_sb = cpool.tile([128, 8], F32, tag="thrk")
            nc.sync.dma_start(thrk_sb[:], thrk[:, :])
            thrvb_sb = cpool.tile([128, D], F32, tag="thrvb")
            nc.sync.dma_start(thrvb_sb[:], thrvb[:, :])
            throb_sb = cpool.tile([128, D], F32, tag="throb")
            nc.sync.dma_start(throb_sb[:], throb[:, :])
            # ones rows at partition bases 0 and 32 for the K=1 broadcast mms
            ones_sb = cpool.tile([33, 64], F32, tag="ones")
            nc.vector.memset(ones_sb[:], 1.0)
            nbias = cpool.tile([128, 1], F32, tag="nbias")
            nc.vector.memset(nbias[:], -6.0)

            # ---- load x (full batch + own shard) and weights, chunked so the
            # first projection matmuls can start before the full load lands
            xt = bigpool.tile([128, 8, S], FP8, tag="xt")
            xt_view = xT[:, :].rearrange("(c p) t -> p c t", p=128)
            wk_sb = wpool.tile([128, 8, D], FP8, tag="w")
            wk_view = wkT[:, :].rearrange("(c p) f -> p c f", p=128)
            for c2 in range(4):
                nc.sync.dma_start(
                    xt[:, 2 * c2 : 2 * c2 + 2, :], xt_view[:, 2 * c2 : 2 * c2 + 2, :]
                )
                nc.sync.dma_start(
                    wk_sb[:, 2 * c2 : 2 * c2 + 2, :],
                    wk_view[:, 2 * c2 : 2 * c2 + 2, :],
                )
            xqt = bigpool.tile([128, 8, TPC], FP8, tag="xqt")
            nc.sync.dma_start(xqt[:], xqT[:, :].rearrange("(c p) t -> p c t", p=128))
            wv_sb = wpool.tile([128, 8, D], FP8, tag="w")
            nc.sync.dma_start(wv_sb[:], wvT[:, :].rearrange("(c p) f -> p c f", p=128))

            kt_full = bigpool.tile([128, 8, S], FP8, tag="kt")
            v_all = bigpool.tile([128, 16, D], FP8E5, tag="vall")

            # ---- K projection, full batch: KT [1024 f, 2048 t] binary fp8
            # (fp8 DoubleRow: contract 256 per matmul via chunk pairs)
            for jf in range(8):
                for tch in range(4):
                    ps = ps_mm.tile([128, 512], F32, tag="mm")
                    for dc in range(4):
                        nc.tensor.matmul(
                            ps[:],
                            lhsT=wk_sb[:, 2 * dc : 2 * dc + 2, jf * 128 : (jf + 1) * 128],
                            rhs=xt[:, 2 * dc : 2 * dc + 2, tch * 512 : (tch + 1) * 512],
                            start=(dc == 0),
                            stop=(dc == 3),
                            perf_mode=DR,
                        )
                    nc.vector.tensor_scalar(
                        out=kt_full[:, jf, tch * 512 : (tch + 1) * 512],
                        in0=ps[:],
                        scalar1=thrk_sb[:, jf : jf + 1],
                        scalar2=None,
                        op0=GT,
                    )

            # ---- V projection, full batch: V [2048 t, 1024 f] binary fp8e5
            for tt in range(16):
                for fh in range(2):
                    ps = ps_mm.tile([128, 512], F32, tag="mm")
                    for dc in range(4):
                        nc.tensor.matmul(
                            ps[:],
                            lhsT=xt[:, 2 * dc : 2 * dc + 2, tt * 128 : (tt + 1) * 128],
                            rhs=wv_sb[:, 2 * dc : 2 * dc + 2, fh * 512 : (fh + 1) * 512],
                            start=(dc == 0),
                            stop=(dc == 3),
                            perf_mode=DR,
                        )
                    nc.vector.tensor_tensor(
                        out=v_all[:, tt, fh * 512 : (fh + 1) * 512],
                        in0=ps[:],
                        in1=thrvb_sb[:, fh * 512 : (fh + 1) * 512],
                        op=GT,
                    )

            # ---- Q projection (own 512 tokens): QT [1024 f, 512 t] binary fp8
            wq_sb = wpool.tile([128, 8, D], FP8, tag="w")
            nc.sync.dma_start(wq_sb[:], wqT[:, :].rearrange("(c p) f -> p c f", p=128))
            qt = bigpool.tile([128, 8, TPC], FP8, tag="qt")
            for jf in range(8):
                ps = ps_mm.tile([128, 512], F32, tag="mm")
                for dc in range(4):
                    nc.tensor.matmul(
                        ps[:],
                        lhsT=wq_sb[:, 2 * dc : 2 * dc + 2, jf * 128 : (jf + 1) * 128],
                        rhs=xqt[:, 2 * dc : 2 * dc + 2, :],
                        start=(dc == 0),
                        stop=(dc == 3),
                        perf_mode=DR,
                    )
                nc.vector.tensor_scalar(
                    out=qt[:, jf, :],
                    in0=ps[:],
                    scalar1=thrq_sb[:, jf : jf + 1],
                    scalar2=None,
                    op0=GT,
                )

            wo_sb = wopool.tile([128, 8, D], FP8, tag="wo")
            nc.sync.dma_start(wo_sb[:], woT[:, :].rearrange("(c p) f -> p c f", p=128))

            at = bigpool.tile([128, 8, TPC], FP8, tag="at")

            # ---- attention; head pair jj = heads (2jj, 2jj+1)
            for jj in range(8):
                au_tiles = []
                den = nrmpool.tile([33, 512], F32, tag="den")
                for hp in range(2):
                    h = 2 * jj + hp
                    # V' per head: 64 value cols + ones col, padded to stride
                    # 80 (DoubleRow needs middle-dim step % 16 == 0)
                    vh = vhpool.tile([128, 16, 80], FP8E5, tag="vh")
                    nc.vector.memset(vh[:, :, 64:65], 1.0)
                    nc.vector.tensor_copy(
                        vh[:, :, 0:64], v_all[:, :, h * 64 : (h + 1) * 64]
                    )
                    p_tiles = []
                    for g in range(8):  # score groups of 2 ktiles, dbl-buffered
                        sc = ps_s.tile([128, 1024], F32, tag="sc")
                        for s2 in range(2):
                            kcc = 2 * g + s2
                            nc.tensor.matmul(
                                sc[:, s2 * 512 : (s2 + 1) * 512],
                                lhsT=kt_full[
                                    hp * 64 : (hp + 1) * 64,
                                    jj,
                                    kcc * 128 : (kcc + 1) * 128,
                                ],
                                rhs=qt[hp * 64 : (hp + 1) * 64, jj, :],
                                start=True,
                                stop=True,
                            )
                        # p = exp(dot/4 - 6); squared-softmax == softmax(2s),
                        # the e^-6 shift cancels in the normalization and
                        # keeps p inside fp8e5's range [2^-16, 57344]
                        p_g = ppool.tile([128, 2, 512], FP8E5, tag="p")
                        nc.scalar.activation(
                            p_g[:, :, :].rearrange("p a b -> p (a b)"),
                            sc[:],
                            AF.Exp,
                            bias=nbias[:],
                            scale=0.25,
                        )
                        p_tiles.append(p_g)
                    pv = ps_pv.tile([65, 512], F32, tag="pv")
                    for kc in range(16):
                        nc.tensor.matmul(
                            pv[:],
                            lhsT=vh[:, kc, 0:65],
                            rhs=p_tiles[kc // 2][:, kc % 2, :],
                            start=(kc == 0),
                            stop=(kc == 15),
                        )
                    # copy numerator + denominator off PSUM immediately so the
                    # slot frees for the next head's PV (keeps PE gap-free)
                    nc.vector.tensor_copy(den[32 * hp : 32 * hp + 1, :], pv[64:65, :])
                    au = aupool.tile([64, 512], BF16, tag="au")
                    nc.vector.tensor_copy(au[:], pv[0:64, :])
                    au_tiles.append(au)
                rec = nrmpool.tile([33, 512], F32, tag="rec")
                nc.vector.reciprocal(rec[:], den[:])
                for hp in range(2):
                    bpp = ps_mm.tile([128, 512], F32, tag="mm")
                    nc.tensor.matmul(
                        bpp[0:64, :],
                        lhsT=ones_sb[32 * hp : 32 * hp + 1, :],
                        rhs=rec[32 * hp : 32 * hp + 1, :],
                        start=True,
                        stop=True,
                    )
                    bps = bppool.tile([64, 512], F32, tag="bp")
                    nc.vector.tensor_copy(bps[:], bpp[0:64, :])
                    nc.vector.tensor_tensor(
                        out=at[hp * 64 : (hp + 1) * 64, jj, :],
                        in0=au_tiles[hp][:],
                        in1=bps[:],
                        op=MULT,
                    )

            # ---- output projection + threshold
            for tt in range(4):
                for fh in range(2):
                    ps = ps_mm.tile([128, 512], F32, tag="mm")
                    for j2 in range(4):
                        nc.tensor.matmul(
                            ps[:],
                            lhsT=at[:, 2 * j2 : 2 * j2 + 2, tt * 128 : (tt + 1) * 128],
                            rhs=wo_sb[:, 2 * j2 : 2 * j2 + 2, fh * 512 : (fh + 1) * 512],
                            start=(j2 == 0),
                            stop=(j2 == 3),
                            perf_mode=DR,
                        )
                    ys = yopool.tile([128, 512], F32, tag="y")
                    nc.vector.tensor_tensor(
                        out=ys[:],
                        in0=ps[:],
                        in1=throb_sb[:, fh * 512 : (fh + 1) * 512],
                        op=GT,
                    )
                    nc.sync.dma_start(
                        y[tt * 128 : (tt + 1) * 128, fh * 512 : (fh + 1) * 512], ys[:]
                    )

    nc.compile()
    return nc


def _get_program():
    if "nc" not in _CACHE:
        _CACHE["nc"] = _build_program()
    return _CACHE["nc"]


def _prep_inputs(x, wq, bq, wk, bk, wv, bv, wo, bo):
    bf16 = ml_dtypes.bfloat16
    fp8 = ml_dtypes.float8_e4m3
    x = np.asarray(x, dtype=np.float32)

    def binT(w, dt):
        bw = np.clip(np.sign(np.asarray(w, dtype=np.float32)), 0.0, 1.0)
        return np.ascontiguousarray(bw.T).astype(dt)

    shared = {
        "wqT": binT(wq, fp8),
        "wkT": binT(wk, fp8),
        "wvT": binT(wv, fp8),
        "woT": binT(wo, fp8),
        "thrq": np.ascontiguousarray(
            (0.5 - np.asarray(bq, np.float32)).reshape(8, 128).T
        ),
        "thrk": np.ascontiguousarray(
            (0.5 - np.asarray(bk, np.float32)).reshape(8, 128).T
        ),
        "thrvb": np.ascontiguousarray(
            np.tile((0.5 - np.asarray(bv, np.float32))[None, :], (128, 1))
        ),
        "throb": np.ascontiguousarray(
            np.tile((0.5 - np.asarray(bo, np.float32))[None, :], (128, 1))
        ),
    }
    in_maps = []
    for c in range(NCORES):
        b, blk = c // 4, c % 4
        xT_b = np.ascontiguousarray(x[b].T).astype(fp8)
        m = dict(shared)
        m["xT"] = xT_b
        m["xqT"] = np.ascontiguousarray(xT_b[:, blk * TPC : (blk + 1) * TPC])
        in_maps.append(m)
    return in_maps


def _gather_output(results):
    y = np.empty((B, S, D), dtype=np.float32)
    for c in range(NCORES):
        b, blk = c // 4, c % 4
        y[b, blk * TPC : (blk + 1) * TPC, :] = results[c]["y"]
    return y


def _run(in_maps, **kw):
    from concourse.bass_utils import run_bass_kernel_spmd

    nc = _get_program()
    return run_bass_kernel_spmd(nc, in_maps, list(range(NCORES)), **kw)


def kernel(x, wq, bq, wk, bk, wv, bv, wo, bo):
    in_maps = _prep_inputs(x, wq, bq, wk, bk, wv, bv, wo, bo)
    res = _run(in_maps)
    return _gather_output(res.results)


def run_traced(inputs, **kw):
    """For test.py: run with NTFF tracing, return (output, BassKernelResults)."""
    in_maps = _prep_inputs(**inputs)
    res = _run(in_maps, trace=True, **kw)
    return _gather_output(res.results), res
